# revision 14
# baseline (speedup 1.0000x reference)
"""Trainium2 Bass kernel for CrossAttention (GQA, causal + key-padding mask).

Problem (hardcoded): q [2,2048,16,64] f32, kv [2,2048,2,4,64] f32,
key_padding_mask [2,2048] bool -> out [2,2048,16,64] f32.

Sharding: 8 cores = (batch b in 0..1) x (kv-head j in 0..3).  Each core
computes the 4 query heads 4j..4j+3 (GQA group) against its single kv head
-- fully independent, no collectives.

Per-core kernel (transpose-free layout, S^T = [k, q]):
  S^T[k_blk, q]  = kT_blk.T @ qT            (PE, f32r)
  E = exp(S^T * scale + pad_k)              (ACT, pad folded into the bias,
                                             only causally-visible columns)
  causal fill: memset fully-masked column range to 0, multiply the diagonal
  [128,128] quarter by a static lower-triangle 0/1 mask (DVE)
  O_aug^T[65, q] += V_blk_aug.T @ E         (PE; V augmented with a ones
                                             column -> row 64 = softmax denom)
  copy PSUM->SBUF (DVE), DMA out (SP/HWDGE)

Host: shards/transposes inputs, divides by the denominator row, transposes
back, and patches "dead" rows (t < first valid key) exactly in numpy.
"""

import sys

sys.path.insert(0, "/opt/trn_rl_repo")

import math
from collections import deque
from contextlib import ExitStack

import numpy as np

import concourse.bass as bass
import concourse.tile as tile
from concourse import mybir
from concourse.bass_utils import run_bass_kernel_spmd

B, SQ, SK, H, HKV, D = 2, 2048, 2048, 16, 4, 64
G = H // HKV  # q heads per core
NCORES = B * HKV
QC = 1024  # q chunk width per unit
MM = 512  # matmul moving-dim width (f32r max)
KB = 128  # k block (partition dim)
NQC = SQ // QC
NKB = SK // KB
RB = QC // KB  # k blocks per q chunk (diagonal band width)
NEG = -10000.0
SCALE = 1.0 / math.sqrt(D)
F32 = mybir.dt.float32
F32R = mybir.dt.float32r
PV_LAG = 2  # software pipeline depth for the PV matmul


def build_nc(reps=1):
    nc = bass.Bass("TRN2")
    qT_d = nc.declare_dram_parameter("qT", [D, G, SQ], F32R, isOutput=False)
    kT_d = nc.declare_dram_parameter("kT", [D, SK], F32R, isOutput=False)
    v_d = nc.declare_dram_parameter("v", [KB, NKB, D + 1], F32R, isOutput=False)
    pad_d = nc.declare_dram_parameter("pad", [KB, NKB], F32, isOutput=False)
    tri_d = nc.declare_dram_parameter("tri", [KB, KB], F32R, isOutput=False)
    o_d = nc.declare_dram_parameter("o", [G, NQC, D + 1, QC], F32, isOutput=True)

    with ExitStack() as ctx:
        tc = ctx.enter_context(tile.TileContext(nc))
        ins = ctx.enter_context(tc.tile_pool(name="ins", bufs=1))
        spool = ctx.enter_context(tc.tile_pool(name="s", bufs=2, space="PSUM"))
        opool = ctx.enter_context(tc.tile_pool(name="op", bufs=2, space="PSUM"))
        epool = ctx.enter_context(tc.tile_pool(name="e", bufs=4))
        obuf = ctx.enter_context(tc.tile_pool(name="ob", bufs=2))

        v_sb = ins.tile([KB, NKB, D + 1], F32R)
        nc.sync.dma_start(out=v_sb[:], in_=v_d[:])
        pad_sb = ins.tile([KB, NKB], F32)
        nc.sync.dma_start(out=pad_sb[:], in_=pad_d[:])
        tri_sb = ins.tile([KB, KB], F32R)
        nc.sync.dma_start(out=tri_sb[:], in_=tri_d[:])
        qT_sb = ins.tile([D, G, SQ], F32R)
        nc.sync.dma_start(out=qT_sb[:], in_=qT_d[:])
        kT_sb = ins.tile([D, SK], F32R)
        nc.sync.dma_start(out=kT_sb[:], in_=kT_d[:])

        pend = deque()  # deferred PV matmuls + epilogues (software pipeline)

        def flush(limit):
            while len(pend) > limit:
                pend.popleft()()

        for rep in range(reps):
            for qc in range(NQC):
                for h in range(G):
                    _one_unit(nc, qc, h, pend, flush, spool, opool, epool, obuf,
                              qT_sb, kT_sb, v_sb, pad_sb, tri_sb, o_d)
        flush(0)
    _split_excess_waits(nc)
    return nc


def _one_unit(nc, qc, h, pend, flush, spool, opool, epool, obuf,
              qT_sb, kT_sb, v_sb, pad_sb, tri_sb, o_d):
                nkb = (qc + 1) * RB  # causal: k blocks 0..nkb-1 visible
                o_ps = opool.tile([D + 1, QC], F32, tag="op")
                for kb in range(nkb):
                    r = kb - qc * RB  # >=0 on the diagonal band
                    lo = KB * r if r > 0 else 0  # cols < lo fully masked
                    h0 = lo // MM  # first 512-half with visible columns
                    s_ps = spool.tile([KB, QC], F32, tag="s")
                    for half in range(h0, QC // MM):
                        nc.tensor.matmul(
                            s_ps[:, half * MM : (half + 1) * MM],
                            lhsT=kT_sb[:, kb * KB : (kb + 1) * KB],
                            rhs=qT_sb[:, h, qc * QC + half * MM : qc * QC + (half + 1) * MM],
                            start=True,
                            stop=True,
                        )
                    e_sb = epool.tile([KB, QC], F32R, tag="e")
                    nc.scalar.activation(
                        out=e_sb[:, lo:],
                        in_=s_ps[:, lo:],
                        func=mybir.ActivationFunctionType.Exp,
                        bias=pad_sb[:, kb : kb + 1],
                        scale=SCALE,
                    )
                    if r >= 0:
                        # zero the fully-masked cols inside the first used half
                        if lo > h0 * MM:
                            nc.vector.memset(e_sb[:, h0 * MM : lo].bitcast(F32), 0.0)
                        # mask the diagonal [128,128] quarter: keep j >= p
                        nc.vector.tensor_mul(
                            e_sb[:, lo : lo + KB],
                            e_sb[:, lo : lo + KB],
                            tri_sb[:],
                        )

                    def pv(o_ps=o_ps, kb=kb, e_sb=e_sb, h0=h0, st=(kb == 0), sp=(kb == nkb - 1)):
                        for half in range(h0, QC // MM):
                            nc.tensor.matmul(
                                o_ps[:, half * MM : (half + 1) * MM],
                                lhsT=v_sb[:, kb, :],
                                rhs=e_sb[:, half * MM : (half + 1) * MM],
                                start=st,
                                stop=sp,
                                skip_group_check=True,
                            )

                    pend.append(pv)
                    flush(PV_LAG)

                def epilogue(o_ps=o_ps, h=h, qc=qc):
                    ob = obuf.tile([D + 1, QC], F32, tag="ob")
                    nc.vector.tensor_copy(out=ob[:], in_=o_ps[:])
                    nc.sync.dma_start(out=o_d[h, qc], in_=ob[:])

                pend.append(epilogue)


def _split_excess_waits(nc, mm_cap=1, default_cap=1):
    """walrus codegen caps sem waits per instruction (1 on self-loading f32r
    matmuls, 1 on ACT, ~4 on drains).  Move excess on_wait entries onto
    standalone InstEventSemaphore instructions inserted just before the owner
    -- engines execute their stream in order, so semantics are identical."""
    n = 0
    for bb in nc.m.functions[0].blocks:
        out = []
        for i in bb.instructions:
            si = i.sync_info
            waits = list(si.on_wait) if (si is not None and si.on_wait) else []
            cap = mm_cap if type(i).__name__ == "InstMatmult" else default_cap
            if len(waits) > cap:
                split_at = len(waits) - cap
                for w in waits[:split_at]:
                    n += 1
                    ws = mybir.InstEventSemaphore(name=f"WSPLIT-{n}", ins=[], outs=[])
                    ws.engine = i.engine
                    ws.sync_info = mybir.SyncInfo(on_wait=[w], on_update=[])
                    out.append(ws)
                si.on_wait = waits[split_at:]
            out.append(i)
        bb.instructions = out


def _host_prep(q, kv, key_padding_mask):
    q = np.asarray(q, dtype=np.float32)
    kv = np.asarray(kv, dtype=np.float32)
    mask = np.asarray(key_padding_mask)
    tri = np.tril(np.ones((KB, KB), np.float32)).T.copy()  # keep j >= p
    in_maps = []
    for c in range(NCORES):
        b, j = divmod(c, HKV)
        qT = np.ascontiguousarray(
            q[b].transpose(2, 1, 0)[:, G * j : G * (j + 1), :]
        )  # [D, G, SQ]
        kT = np.ascontiguousarray(kv[b, :, 0, j, :].T)  # [D, SK]
        v_aug = np.concatenate(
            [kv[b, :, 1, j, :], np.ones((SK, 1), np.float32)], axis=1
        )  # [SK, D+1]
        v_blk = np.ascontiguousarray(
            v_aug.reshape(NKB, KB, D + 1).transpose(1, 0, 2)
        )  # [KB, NKB, D+1]
        pad = np.where(mask[b], 0.0, NEG).astype(np.float32)
        pad_blk = np.ascontiguousarray(pad.reshape(NKB, KB).T)  # [KB, NKB]
        in_maps.append({"qT": qT, "kT": kT, "v": v_blk, "pad": pad_blk, "tri": tri})
    return in_maps


def _host_finish(results, q, kv, key_padding_mask):
    q = np.asarray(q, dtype=np.float32)
    kv = np.asarray(kv, dtype=np.float32)
    mask = np.asarray(key_padding_mask)
    out = np.empty((B, SQ, H, D), np.float32)
    for c in range(NCORES):
        b, j = divmod(c, HKV)
        o = results[c]["o"]  # [G, NQC, D+1, QC]
        num = o[:, :, :D, :]  # [G, NQC, D, QC]
        den = o[:, :, D : D + 1, :]  # [G, NQC, 1, QC]
        with np.errstate(divide="ignore", invalid="ignore"):
            norm = num / den
        # [G, NQC, D, QC] -> [SQ, G, D]
        out[b, :, G * j : G * (j + 1), :] = norm.transpose(1, 3, 0, 2).reshape(
            SQ, G, D
        )
    # Patch "dead" rows (no valid key in the causal window): softmax over
    # all-masked scores.  Exact reference semantics, computed on host.
    for b in range(B):
        valid = mask[b]
        fv = int(np.argmax(valid)) if valid.any() else SK
        if fv == 0:
            continue
        pad = np.where(valid, 0.0, NEG).astype(np.float32)
        for t in range(fv):
            for h in range(H):
                j = h // G
                s = (q[b, t, h] @ kv[b, :, 0, j, :].T) * SCALE + pad
                s[t + 1 :] = NEG  # causal: replace, not add
                s -= s.max()
                e = np.exp(s)
                w = e / e.sum()
                out[b, t, h, :] = w @ kv[b, :, 1, j, :]
    return out


_NC_CACHE = None


def kernel(q, kv, key_padding_mask):
    global _NC_CACHE
    if _NC_CACHE is None:
        _NC_CACHE = build_nc()
    in_maps = _host_prep(q, kv, key_padding_mask)
    results = run_bass_kernel_spmd(_NC_CACHE, in_maps, list(range(NCORES))).results
    return _host_finish(results, q, kv, key_padding_mask)


# revision 23
# speedup vs baseline: 772.5519x; 772.5519x over previous
"""Trainium2 Bass kernel for CrossAttention (GQA, causal + key-padding mask).

Problem (hardcoded): q [2,2048,16,64] f32, kv [2,2048,2,4,64] f32,
key_padding_mask [2,2048] bool -> out [2,2048,16,64] f32.

Sharding: 8 cores = (batch b in 0..1) x (kv-head j in 0..3).  Each core
computes the 4 query heads 4j..4j+3 (GQA group) against its single kv head
-- fully independent, no collectives.

Per-core kernel (transpose-free layout, S^T = [k, q]):
  S^T[k_blk, q]  = kT_blk.T @ qT            (PE, f32r)
  E = exp(S^T * scale + pad_k)              (ACT, pad folded into the bias,
                                             only causally-visible columns)
  causal fill: memset fully-masked column range to 0, multiply the diagonal
  [128,128] quarter by a static lower-triangle 0/1 mask (DVE)
  O_aug^T[65, q] += V_blk_aug.T @ E         (PE; V augmented with a ones
                                             column -> row 64 = softmax denom)
  copy PSUM->SBUF (DVE), DMA out (SP/HWDGE)

Host: shards/transposes inputs, divides by the denominator row, transposes
back, and patches "dead" rows (t < first valid key) exactly in numpy.
"""

import sys

sys.path.insert(0, "/opt/trn_rl_repo")

import math
from collections import deque
from contextlib import ExitStack

import numpy as np

import concourse.bass as bass
import concourse.tile as tile
from concourse import mybir
from concourse.bass_utils import run_bass_kernel_spmd

B, SQ, SK, H, HKV, D = 2, 2048, 2048, 16, 4, 64
G = H // HKV  # q heads per core
NCORES = B * HKV
QC = 1024  # q chunk width per unit
MM = 512  # matmul moving-dim width (f32r max)
KB = 128  # k block (partition dim)
NQC = SQ // QC
NKB = SK // KB
RB = QC // KB  # k blocks per q chunk (diagonal band width)
NEG = -10000.0
SCALE = 1.0 / math.sqrt(D)
F32 = mybir.dt.float32
F32R = mybir.dt.float32r
PV_LAG = 2  # software pipeline depth for the PV matmul


def build_nc(reps=1):
    nc = bass.Bass("TRN2")
    qT_d = nc.declare_dram_parameter("qT", [D, G, SQ], F32R, isOutput=False)
    kT_d = nc.declare_dram_parameter("kT", [D, SK], F32R, isOutput=False)
    v_d = nc.declare_dram_parameter("v", [KB, NKB, D + 1], F32R, isOutput=False)
    ptri_d = nc.declare_dram_parameter("ptri", [KB, NKB + KB], F32, isOutput=False)
    o_d = nc.declare_dram_parameter("o", [G, NQC, D + 1, QC], F32, isOutput=True)

    with ExitStack() as ctx:
        tc = ctx.enter_context(tile.TileContext(nc))
        ins = ctx.enter_context(tc.tile_pool(name="ins", bufs=1))
        spool = ctx.enter_context(tc.tile_pool(name="s", bufs=3, space="PSUM"))
        opool = ctx.enter_context(tc.tile_pool(name="op", bufs=1, space="PSUM"))
        epool = ctx.enter_context(tc.tile_pool(name="e", bufs=6))
        obuf = ctx.enter_context(tc.tile_pool(name="ob", bufs=3))

        # Stream inputs in first-use order (HWDGE issue is ~0.65us each, so
        # few-but-ordered DMAs): unit (0,0) needs ptri + kT/qT/v first halves.
        ptri_sb = ins.tile([KB, NKB + KB], F32)
        pad_sb = ptri_sb[:, :NKB]
        tri_sb = ptri_sb[:, NKB:].bitcast(F32R)
        kT_sb = ins.tile([D, SK], F32R)
        qT_sb = ins.tile([D, G, SQ], F32R)
        v_sb = ins.tile([KB, NKB, D + 1], F32R)
        nc.sync.dma_start(out=kT_sb[:, :QC], in_=kT_d[:, :QC])
        nc.sync.dma_start(out=qT_sb[:, 0, :QC], in_=qT_d[:, 0, :QC])
        nc.sync.dma_start(out=ptri_sb[:], in_=ptri_d[:])
        nc.sync.dma_start(out=v_sb[:, :RB], in_=v_d[:, :RB])
        nc.sync.dma_start(out=qT_sb[:, 1:, :QC], in_=qT_d[:, 1:, :QC])
        nc.sync.dma_start(out=kT_sb[:, QC:], in_=kT_d[:, QC:])
        nc.sync.dma_start(out=v_sb[:, RB:], in_=v_d[:, RB:])
        nc.sync.dma_start(out=qT_sb[:, :, QC:], in_=qT_d[:, :, QC:])

        pend = deque()  # deferred PV matmuls + epilogues (software pipeline)

        def flush(limit):
            while len(pend) > limit:
                pend.popleft()()

        for rep in range(reps):
            for qc in range(NQC):
                for h in range(G):
                    _one_unit(nc, qc, h, pend, flush, spool, opool, epool, obuf,
                              qT_sb, kT_sb, v_sb, pad_sb, tri_sb, o_d)
        flush(0)
    _split_excess_waits(nc)
    return nc


def _one_unit(nc, qc, h, pend, flush, spool, opool, epool, obuf,
              qT_sb, kT_sb, v_sb, pad_sb, tri_sb, o_d):
                nkb = (qc + 1) * RB  # causal: k blocks 0..nkb-1 visible
                o_ps = opool.tile([D + 1, QC], F32, tag="op")
                for kb in range(nkb):
                    r = kb - qc * RB  # >=0 on the diagonal band
                    lo = KB * r if r > 0 else 0  # cols < lo fully masked
                    h0 = lo // MM  # first 512-half with visible columns
                    # first-half matmuls start at qs: skip fully-masked cols
                    # but keep moving dim >= 256 (f32r full speed needs it)
                    qs = h0 * MM + min(lo - h0 * MM, MM - 256)
                    s_ps = spool.tile([KB, QC], F32, tag="s")
                    for half in range(h0, QC // MM):
                        c0 = qs if half == h0 else half * MM
                        nc.tensor.matmul(
                            s_ps[:, c0 : (half + 1) * MM],
                            lhsT=kT_sb[:, kb * KB : (kb + 1) * KB],
                            rhs=qT_sb[:, h, qc * QC + c0 : qc * QC + (half + 1) * MM],
                            start=True,
                            stop=True,
                        )
                    e_sb = epool.tile([KB, QC], F32R, tag="e")
                    nc.scalar.activation(
                        out=e_sb[:, lo:],
                        in_=s_ps[:, lo:],
                        func=mybir.ActivationFunctionType.Exp,
                        bias=pad_sb[:, kb : kb + 1],
                        scale=SCALE,
                    )
                    if r >= 0:
                        # zero the fully-masked cols inside the first used half
                        if lo > qs:
                            nc.vector.memset(e_sb[:, qs:lo].bitcast(F32), 0.0)
                        # mask the diagonal [128,128] quarter: keep j >= p
                        nc.vector.tensor_mul(
                            e_sb[:, lo : lo + KB],
                            e_sb[:, lo : lo + KB],
                            tri_sb[:],
                        )

                    def pv(o_ps=o_ps, kb=kb, e_sb=e_sb, h0=h0, qs=qs, st=(kb == 0), sp=(kb == nkb - 1)):
                        for half in range(h0, QC // MM):
                            c0 = qs if half == h0 else half * MM
                            nc.tensor.matmul(
                                o_ps[:, c0 : (half + 1) * MM],
                                lhsT=v_sb[:, kb, :],
                                rhs=e_sb[:, c0 : (half + 1) * MM],
                                start=st,
                                stop=sp,
                                skip_group_check=True,
                            )

                    pend.append(pv)
                    flush(PV_LAG)

                def epilogue(o_ps=o_ps, h=h, qc=qc):
                    ob = obuf.tile([D + 1, QC], F32, tag="ob")
                    nc.vector.tensor_copy(out=ob[:], in_=o_ps[:])
                    nc.sync.dma_start(out=o_d[h, qc], in_=ob[:])

                pend.append(epilogue)


def _split_excess_waits(nc, mm_cap=1, default_cap=1):
    """walrus codegen caps sem waits per instruction (1 on self-loading f32r
    matmuls, 1 on ACT, ~4 on drains).  Move excess on_wait entries onto
    standalone InstEventSemaphore instructions inserted just before the owner
    -- engines execute their stream in order, so semantics are identical."""
    n = 0
    for bb in nc.m.functions[0].blocks:
        out = []
        for i in bb.instructions:
            si = i.sync_info
            waits = list(si.on_wait) if (si is not None and si.on_wait) else []
            cap = mm_cap if type(i).__name__ == "InstMatmult" else default_cap
            if len(waits) > cap:
                split_at = len(waits) - cap
                for w in waits[:split_at]:
                    n += 1
                    ws = mybir.InstEventSemaphore(name=f"WSPLIT-{n}", ins=[], outs=[])
                    ws.engine = i.engine
                    ws.sync_info = mybir.SyncInfo(on_wait=[w], on_update=[])
                    out.append(ws)
                si.on_wait = waits[split_at:]
            out.append(i)
        bb.instructions = out


def _host_prep(q, kv, key_padding_mask):
    q = np.asarray(q, dtype=np.float32)
    kv = np.asarray(kv, dtype=np.float32)
    mask = np.asarray(key_padding_mask)
    tri = np.tril(np.ones((KB, KB), np.float32)).T.copy()  # keep j >= p
    in_maps = []
    for c in range(NCORES):
        b, j = divmod(c, HKV)
        qT = np.ascontiguousarray(
            q[b].transpose(2, 1, 0)[:, G * j : G * (j + 1), :]
        )  # [D, G, SQ]
        kT = np.ascontiguousarray(kv[b, :, 0, j, :].T)  # [D, SK]
        v_aug = np.concatenate(
            [kv[b, :, 1, j, :], np.ones((SK, 1), np.float32)], axis=1
        )  # [SK, D+1]
        v_blk = np.ascontiguousarray(
            v_aug.reshape(NKB, KB, D + 1).transpose(1, 0, 2)
        )  # [KB, NKB, D+1]
        pad = np.where(mask[b], 0.0, NEG).astype(np.float32)
        pad_blk = np.ascontiguousarray(pad.reshape(NKB, KB).T)  # [KB, NKB]
        ptri = np.concatenate([pad_blk, tri], axis=1)  # [KB, NKB+KB]
        in_maps.append({"qT": qT, "kT": kT, "v": v_blk, "ptri": ptri})
    return in_maps


def _host_finish(results, q, kv, key_padding_mask):
    q = np.asarray(q, dtype=np.float32)
    kv = np.asarray(kv, dtype=np.float32)
    mask = np.asarray(key_padding_mask)
    out = np.empty((B, SQ, H, D), np.float32)
    for c in range(NCORES):
        b, j = divmod(c, HKV)
        o = results[c]["o"]  # [G, NQC, D+1, QC]
        num = o[:, :, :D, :]  # [G, NQC, D, QC]
        den = o[:, :, D : D + 1, :]  # [G, NQC, 1, QC]
        with np.errstate(divide="ignore", invalid="ignore"):
            norm = num / den
        # [G, NQC, D, QC] -> [SQ, G, D]
        out[b, :, G * j : G * (j + 1), :] = norm.transpose(1, 3, 0, 2).reshape(
            SQ, G, D
        )
    # Patch "dead" rows (no valid key in the causal window): softmax over
    # all-masked scores.  Exact reference semantics, computed on host.
    for b in range(B):
        valid = mask[b]
        fv = int(np.argmax(valid)) if valid.any() else SK
        if fv == 0:
            continue
        pad = np.where(valid, 0.0, NEG).astype(np.float32)
        for t in range(fv):
            for h in range(H):
                j = h // G
                s = (q[b, t, h] @ kv[b, :, 0, j, :].T) * SCALE + pad
                s[t + 1 :] = NEG  # causal: replace, not add
                s -= s.max()
                e = np.exp(s)
                w = e / e.sum()
                out[b, t, h, :] = w @ kv[b, :, 1, j, :]
    return out


_NC_CACHE = None


def kernel(q, kv, key_padding_mask):
    global _NC_CACHE
    if _NC_CACHE is None:
        _NC_CACHE = build_nc()
    in_maps = _host_prep(q, kv, key_padding_mask)
    results = run_bass_kernel_spmd(_NC_CACHE, in_maps, list(range(NCORES))).results
    return _host_finish(results, q, kv, key_padding_mask)


# revision 26
# speedup vs baseline: 780.0530x; 1.0097x over previous
"""Trainium2 Bass kernel for CrossAttention (GQA, causal + key-padding mask).

Problem (hardcoded): q [2,2048,16,64] f32, kv [2,2048,2,4,64] f32,
key_padding_mask [2,2048] bool -> out [2,2048,16,64] f32.

Sharding: 8 cores = (batch b in 0..1) x (kv-head j in 0..3).  Each core
computes the 4 query heads 4j..4j+3 (GQA group) against its single kv head
-- fully independent, no collectives.

Per-core kernel (transpose-free layout, S^T = [k, q]):
  S^T[k_blk, q]  = kT_blk.T @ qT            (PE, f32r)
  E = exp(S^T * scale + pad_k)              (ACT, pad folded into the bias,
                                             only causally-visible columns)
  causal fill: memset fully-masked column range to 0, multiply the diagonal
  [128,128] quarter by a static lower-triangle 0/1 mask (DVE)
  O_aug^T[65, q] += V_blk_aug.T @ E         (PE; V augmented with a ones
                                             column -> row 64 = softmax denom)
  copy PSUM->SBUF (DVE), DMA out (SP/HWDGE)

Host: shards/transposes inputs, divides by the denominator row, transposes
back, and patches "dead" rows (t < first valid key) exactly in numpy.
"""

import sys

sys.path.insert(0, "/opt/trn_rl_repo")

import math
from collections import deque
from contextlib import ExitStack

import numpy as np

import concourse.bass as bass
import concourse.tile as tile
from concourse import mybir
from concourse.bass_utils import run_bass_kernel_spmd

B, SQ, SK, H, HKV, D = 2, 2048, 2048, 16, 4, 64
G = H // HKV  # q heads per core
NCORES = B * HKV
QC = 1024  # q chunk width per unit
MM = 512  # matmul moving-dim width (f32r max)
KB = 128  # k block (partition dim)
NQC = SQ // QC
NKB = SK // KB
RB = QC // KB  # k blocks per q chunk (diagonal band width)
NEG = -10000.0
SCALE = 1.0 / math.sqrt(D)
F32 = mybir.dt.float32
F32R = mybir.dt.float32r
PV_LAG = 2  # software pipeline depth for the PV matmul


def build_nc(reps=1):
    nc = bass.Bass("TRN2")
    qT_d = nc.declare_dram_parameter("qT", [D, G, SQ], F32R, isOutput=False)
    kT_d = nc.declare_dram_parameter("kT", [D, SK], F32R, isOutput=False)
    v_d = nc.declare_dram_parameter("v", [KB, NKB, D + 1], F32R, isOutput=False)
    ptri_d = nc.declare_dram_parameter("ptri", [KB, NKB + KB], F32, isOutput=False)
    o_d = nc.declare_dram_parameter("o", [G, NQC, D + 1, QC], F32, isOutput=True)

    with ExitStack() as ctx:
        tc = ctx.enter_context(tile.TileContext(nc))
        ins = ctx.enter_context(tc.tile_pool(name="ins", bufs=1))
        spool = ctx.enter_context(tc.tile_pool(name="s", bufs=3, space="PSUM"))
        opool = ctx.enter_context(tc.tile_pool(name="op", bufs=1, space="PSUM"))
        epool = ctx.enter_context(tc.tile_pool(name="e", bufs=6))
        obuf = ctx.enter_context(tc.tile_pool(name="ob", bufs=3))

        # Stream inputs in first-use order (HWDGE issue is ~0.65us each, so
        # few-but-ordered DMAs): unit (0,0) needs ptri + kT/qT/v first halves.
        ptri_sb = ins.tile([KB, NKB + KB], F32)
        pad_sb = ptri_sb[:, :NKB]
        tri_sb = ptri_sb[:, NKB:].bitcast(F32R)
        kT_sb = ins.tile([D, SK], F32R)
        qT_sb = ins.tile([D, G, SQ], F32R)
        v_sb = ins.tile([KB, NKB, D + 1], F32R)
        nc.sync.dma_start(out=kT_sb[:, :KB], in_=kT_d[:, :KB])
        nc.sync.dma_start(out=qT_sb[:, 0, :QC], in_=qT_d[:, 0, :QC])
        nc.sync.dma_start(out=kT_sb[:, KB:QC], in_=kT_d[:, KB:QC])
        nc.sync.dma_start(out=ptri_sb[:], in_=ptri_d[:])
        nc.sync.dma_start(out=v_sb[:, :RB], in_=v_d[:, :RB])
        nc.sync.dma_start(out=qT_sb[:, 1:, :QC], in_=qT_d[:, 1:, :QC])
        nc.sync.dma_start(out=kT_sb[:, QC:], in_=kT_d[:, QC:])
        nc.sync.dma_start(out=v_sb[:, RB:], in_=v_d[:, RB:])
        nc.sync.dma_start(out=qT_sb[:, :, QC:], in_=qT_d[:, :, QC:])

        # HAM warmup: dummy bf16 matmuls on zeroed SBUF while the input DMAs
        # stream in, so the first real QKs run at the warm PE clock.
        warm_sb = ins.tile([D, MM], mybir.dt.bfloat16)
        nc.vector.memset(warm_sb[:], 0.0)
        for _ in range(8):
            wp = spool.tile([KB, MM], F32, tag="s")
            nc.tensor.matmul(
                wp[:], lhsT=warm_sb[:, :KB], rhs=warm_sb[:], start=True, stop=True
            )

        pend = deque()  # deferred PV matmuls + epilogues (software pipeline)

        def flush(limit):
            while len(pend) > limit:
                pend.popleft()()

        for rep in range(reps):
            for qc in range(NQC):
                for h in range(G):
                    _one_unit(nc, qc, h, pend, flush, spool, opool, epool, obuf,
                              qT_sb, kT_sb, v_sb, pad_sb, tri_sb, o_d)
        flush(0)
    _split_excess_waits(nc)
    return nc


def _one_unit(nc, qc, h, pend, flush, spool, opool, epool, obuf,
              qT_sb, kT_sb, v_sb, pad_sb, tri_sb, o_d):
                nkb = (qc + 1) * RB  # causal: k blocks 0..nkb-1 visible
                o_ps = opool.tile([D + 1, QC], F32, tag="op")
                for kb in range(nkb):
                    r = kb - qc * RB  # >=0 on the diagonal band
                    lo = KB * r if r > 0 else 0  # cols < lo fully masked
                    h0 = lo // MM  # first 512-half with visible columns
                    # first-half matmuls start at qs: skip fully-masked cols
                    # but keep moving dim >= 256 (f32r full speed needs it)
                    qs = h0 * MM + min(lo - h0 * MM, MM - 256)
                    s_ps = spool.tile([KB, QC], F32, tag="s")
                    for half in range(h0, QC // MM):
                        c0 = qs if half == h0 else half * MM
                        nc.tensor.matmul(
                            s_ps[:, c0 : (half + 1) * MM],
                            lhsT=kT_sb[:, kb * KB : (kb + 1) * KB],
                            rhs=qT_sb[:, h, qc * QC + c0 : qc * QC + (half + 1) * MM],
                            start=True,
                            stop=True,
                        )
                    e_sb = epool.tile([KB, QC], F32R, tag="e")
                    nc.scalar.activation(
                        out=e_sb[:, lo:],
                        in_=s_ps[:, lo:],
                        func=mybir.ActivationFunctionType.Exp,
                        bias=pad_sb[:, kb : kb + 1],
                        scale=SCALE,
                    )
                    if r >= 0:
                        # zero the fully-masked cols inside the first used half
                        if lo > qs:
                            nc.vector.memset(e_sb[:, qs:lo].bitcast(F32), 0.0)
                        # mask the diagonal [128,128] quarter: keep j >= p
                        nc.vector.tensor_mul(
                            e_sb[:, lo : lo + KB],
                            e_sb[:, lo : lo + KB],
                            tri_sb[:],
                        )

                    def pv(o_ps=o_ps, kb=kb, e_sb=e_sb, h0=h0, qs=qs, st=(kb == 0), sp=(kb == nkb - 1)):
                        for half in range(h0, QC // MM):
                            c0 = qs if half == h0 else half * MM
                            nc.tensor.matmul(
                                o_ps[:, c0 : (half + 1) * MM],
                                lhsT=v_sb[:, kb, :],
                                rhs=e_sb[:, c0 : (half + 1) * MM],
                                start=st,
                                stop=sp,
                                skip_group_check=True,
                            )

                    pend.append(pv)
                    if r == RB // 2 - 1:
                        # half0 of o_ps is final (blocks r>=RB/2 skip it):
                        # drain it now, overlapping the remaining PVs
                        def epi0(o_ps=o_ps, h=h, qc=qc):
                            ob = obuf.tile([D + 1, MM], F32, tag="ob")
                            nc.vector.tensor_copy(out=ob[:], in_=o_ps[:, :MM])
                            nc.sync.dma_start(out=o_d[h, qc, :, :MM], in_=ob[:])

                        pend.append(epi0)
                    flush(PV_LAG)

                def epi1(o_ps=o_ps, h=h, qc=qc):
                    ob = obuf.tile([D + 1, MM], F32, tag="ob")
                    nc.vector.tensor_copy(out=ob[:], in_=o_ps[:, MM:])
                    nc.sync.dma_start(out=o_d[h, qc, :, MM:], in_=ob[:])

                pend.append(epi1)


def _split_excess_waits(nc, mm_cap=1, default_cap=1):
    """walrus codegen caps sem waits per instruction (1 on self-loading f32r
    matmuls, 1 on ACT, ~4 on drains).  Move excess on_wait entries onto
    standalone InstEventSemaphore instructions inserted just before the owner
    -- engines execute their stream in order, so semantics are identical."""
    n = 0
    for bb in nc.m.functions[0].blocks:
        out = []
        for i in bb.instructions:
            si = i.sync_info
            waits = list(si.on_wait) if (si is not None and si.on_wait) else []
            cap = mm_cap if type(i).__name__ == "InstMatmult" else default_cap
            if len(waits) > cap:
                split_at = len(waits) - cap
                for w in waits[:split_at]:
                    n += 1
                    ws = mybir.InstEventSemaphore(name=f"WSPLIT-{n}", ins=[], outs=[])
                    ws.engine = i.engine
                    ws.sync_info = mybir.SyncInfo(on_wait=[w], on_update=[])
                    out.append(ws)
                si.on_wait = waits[split_at:]
            out.append(i)
        bb.instructions = out


def _host_prep(q, kv, key_padding_mask):
    q = np.asarray(q, dtype=np.float32)
    kv = np.asarray(kv, dtype=np.float32)
    mask = np.asarray(key_padding_mask)
    tri = np.tril(np.ones((KB, KB), np.float32)).T.copy()  # keep j >= p
    in_maps = []
    for c in range(NCORES):
        b, j = divmod(c, HKV)
        qT = np.ascontiguousarray(
            q[b].transpose(2, 1, 0)[:, G * j : G * (j + 1), :]
        )  # [D, G, SQ]
        kT = np.ascontiguousarray(kv[b, :, 0, j, :].T)  # [D, SK]
        v_aug = np.concatenate(
            [kv[b, :, 1, j, :], np.ones((SK, 1), np.float32)], axis=1
        )  # [SK, D+1]
        v_blk = np.ascontiguousarray(
            v_aug.reshape(NKB, KB, D + 1).transpose(1, 0, 2)
        )  # [KB, NKB, D+1]
        pad = np.where(mask[b], 0.0, NEG).astype(np.float32)
        pad_blk = np.ascontiguousarray(pad.reshape(NKB, KB).T)  # [KB, NKB]
        ptri = np.concatenate([pad_blk, tri], axis=1)  # [KB, NKB+KB]
        in_maps.append({"qT": qT, "kT": kT, "v": v_blk, "ptri": ptri})
    return in_maps


def _host_finish(results, q, kv, key_padding_mask):
    q = np.asarray(q, dtype=np.float32)
    kv = np.asarray(kv, dtype=np.float32)
    mask = np.asarray(key_padding_mask)
    out = np.empty((B, SQ, H, D), np.float32)
    for c in range(NCORES):
        b, j = divmod(c, HKV)
        o = results[c]["o"]  # [G, NQC, D+1, QC]
        num = o[:, :, :D, :]  # [G, NQC, D, QC]
        den = o[:, :, D : D + 1, :]  # [G, NQC, 1, QC]
        with np.errstate(divide="ignore", invalid="ignore"):
            norm = num / den
        # [G, NQC, D, QC] -> [SQ, G, D]
        out[b, :, G * j : G * (j + 1), :] = norm.transpose(1, 3, 0, 2).reshape(
            SQ, G, D
        )
    # Patch "dead" rows (no valid key in the causal window): softmax over
    # all-masked scores.  Exact reference semantics, computed on host.
    for b in range(B):
        valid = mask[b]
        fv = int(np.argmax(valid)) if valid.any() else SK
        if fv == 0:
            continue
        pad = np.where(valid, 0.0, NEG).astype(np.float32)
        for t in range(fv):
            for h in range(H):
                j = h // G
                s = (q[b, t, h] @ kv[b, :, 0, j, :].T) * SCALE + pad
                s[t + 1 :] = NEG  # causal: replace, not add
                s -= s.max()
                e = np.exp(s)
                w = e / e.sum()
                out[b, t, h, :] = w @ kv[b, :, 1, j, :]
    return out


_NC_CACHE = None


def kernel(q, kv, key_padding_mask):
    global _NC_CACHE
    if _NC_CACHE is None:
        _NC_CACHE = build_nc()
    in_maps = _host_prep(q, kv, key_padding_mask)
    results = run_bass_kernel_spmd(_NC_CACHE, in_maps, list(range(NCORES))).results
    return _host_finish(results, q, kv, key_padding_mask)
